# revision 2
# baseline (speedup 1.0000x reference)
"""Multi-head attention (B=2, T=2048, D=1024, H=16) on 8 Trainium2 NeuronCores.

Sharding: tensor-parallel over heads -- core c owns global heads {2c, 2c+1}
(Wq/Wk/Wv column-split, Wo row-split, relpos split along H).  Each core
computes a partial [B, 128, NDC, T] output-projection product; the host sums
the 8 partials.  SPMD: one program, per-core slices in the input maps.

v2 redesign vs the k-outer baseline:
  - q-group-outer flash loop, both heads per chunk: relpos lands in PSUM
    first (fp8 id8 matmul, start=True), then the two heads' score matmuls
    run CONCURRENTLY on disjoint PE row groups (h0 rows 0-63, h1 64-127)
    into adjacent PSUM banks of one [128,1024] tile.
  - ONE merged exp ACTIVATE per chunk over both heads' banks (halves the
    per-instruction 352-cycle ACT overhead vs per-qg exps).
  - P@V lags one chunk behind the score stream so the PE never waits on
    the ACT latency.
  - full-contraction output projection: both heads' normalized attention
    outputs live stacked in one [128, T] tile (head1 is lane-shifted via a
    small SBUF->SBUF DMA), so each out-proj piece is a single
    128-contraction matmul instead of two 64-contraction ones.
  - relpos blocks are host-packed so each (qg, ck) chunk is one contiguous
    [128,1024] fp8 DMA into a small rotating pool, prefetched 2 chunks
    ahead on the sync ring (reloaded per batch; keeps SBUF for x tiles).
  - projections are emitted as 512-col pieces and interleaved into the
    attention stream as fillers with explicit per-(b,qg) deadlines;
    attention starts ~7us in instead of ~50us (HAM stays warm).
  - output stores merged: one [128, NDC*512] DMA per (b, qg).
"""

import sys

for p in ("/opt/trn_rl_repo", "/root/.axon_site/_ro/trn_rl_repo"):
    if p not in sys.path:
        sys.path.insert(0, p)

from collections import deque

import numpy as np
import ml_dtypes

import concourse.bacc as bacc
import concourse.mybir as mybir
import concourse.tile as tile
from concourse.bass_utils import run_bass_kernel_spmd

B, T, D, H = 2, 2048, 1024, 16
DK = D // H          # 64
NCORES = 8
HPC = H // NCORES    # heads per core = 2
QG = 512             # q-group width
NQG = T // QG        # 4
NKC = T // 128       # 16 k-chunks
NDC = D // 128       # 8 d-chunks
NEG = np.float32(-1e30)

F32 = mybir.dt.float32
FP16 = mybir.dt.float16
FP8 = mybir.dt.float8e4

_CACHE = {}

# all (qg, ck) chunks of one batch's attention, in emission order
CHUNKS = [(qg, ck) for qg in range(NQG) for ck in range(4 * qg + 4)]


def _build_program():
    nc = bacc.Bacc("TRN2", target_bir_lowering=False, debug=False,
                   enable_asserts=True)

    d_qT = nc.dram_tensor("qT", [B, D, T], FP16, kind="ExternalInput").ap()
    d_kT = nc.dram_tensor("kT", [B, D, T], FP16, kind="ExternalInput").ap()
    d_vT = nc.dram_tensor("vT", [B, D, T], FP16, kind="ExternalInput").ap()
    # [qg, ck, k-row, h*512 + q]  (both heads side by side, causal -240 baked)
    d_rp = nc.dram_tensor("relposT", [NQG, NKC, 128, 2 * QG], FP8,
                          kind="ExternalInput").ap()
    d_kp = nc.dram_tensor("kpadT", [128, B, NKC], F32,
                          kind="ExternalInput").ap()
    d_wq = nc.dram_tensor("wqT", [128, NDC, 128], FP16,
                          kind="ExternalInput").ap()
    d_wk = nc.dram_tensor("wkT", [128, NDC, 128], FP16,
                          kind="ExternalInput").ap()
    d_wv = nc.dram_tensor("wvT", [128, NDC, 128], FP16,
                          kind="ExternalInput").ap()
    d_wo = nc.dram_tensor("woT", [128, D], FP16, kind="ExternalInput").ap()
    d_id8 = nc.dram_tensor("id8", [128, 128], FP8, kind="ExternalInput").ap()
    # [b, p, c, t]: partial out for dims d = c*128 + p
    d_out = nc.dram_tensor("outT", [B, 128, NDC, T], FP16,
                           kind="ExternalOutput").ap()

    with tile.TileContext(nc) as tc:
        with (
            tc.tile_pool(name="persist", bufs=1) as persist,
            tc.tile_pool(name="xs", bufs=25) as xpool,
            tc.tile_pool(name="xf", bufs=12) as xfpool,
            tc.tile_pool(name="rpp", bufs=6) as rppool,
            tc.tile_pool(name="ee", bufs=4) as epool,
            tc.tile_pool(name="nrm", bufs=2) as nrm,
            tc.tile_pool(name="oc", bufs=2) as ocpool,
            tc.tile_pool(name="tmp", bufs=2) as tmppool,
            tc.tile_pool(name="s2", bufs=2, space="PSUM") as ps2,
            tc.tile_pool(name="ops", bufs=1, space="PSUM") as opsP,
            tc.tile_pool(name="mps", bufs=2, space="PSUM") as miscP,
        ):
            # ---- constants (scalar queue is free until the first exp) ----
            id8 = persist.tile([128, 128], FP8, tag="id8", name="id8")
            nc.scalar.dma_start(out=id8[:], in_=d_id8[:])
            kpad = persist.tile([128, B, NKC], F32, tag="kpad", name="kpad")
            nc.scalar.dma_start(out=kpad[:], in_=d_kp[:])
            ones = persist.tile([128, DK], F32, tag="ones", name="ones")
            nc.vector.memset(ones[:], 1.0)
            ones16 = persist.tile([128, DK], FP16, tag="ones16", name="ones16")
            nc.vector.memset(ones16[:], 1.0)

            w_sb = {}
            for nm, dten in (("q", d_wq), ("k", d_wk), ("v", d_wv)):
                w = persist.tile([128, NDC, 128], FP16, tag=f"w{nm}",
                                 name=f"w{nm}")
                nc.scalar.dma_start(out=w[:], in_=dten[:])
                w_sb[nm] = w
            wo_sb = persist.tile([128, D], FP16, tag="wo", name="wo")
            nc.scalar.dma_start(out=wo_sb[:], in_=d_wo[:])

            qt_sb, kt_sb, at2, vaug = {}, {}, {}, {}
            for b in range(B):
                qt_sb[b] = persist.tile([128, T], FP16, tag=f"qt{b}",
                                        name=f"qt{b}")
                kt_sb[b] = persist.tile([128, T], FP16, tag=f"kt{b}",
                                        name=f"kt{b}")
                at2[b] = persist.tile([128, T], FP16, tag=f"at{b}",
                                      name=f"at{b}")
                for h in range(HPC):
                    va = persist.tile([128, NKC * 80], FP16, tag=f"va{b}{h}",
                                      name=f"va{b}{h}")
                    va_c = va[:].rearrange("p (c u) -> p c u", u=80)
                    nc.vector.tensor_copy(va_c[:, :, DK], ones[:, 0:NKC])
                    vaug[(b, h)] = va

            # ---- x input: b0 as [128,1024] halves (fine startup grain),
            # b1 as [128,2048] full rows (half the DMA-issue count, which
            # otherwise serializes ~60us on the one gpsimd queue).  All x
            # rides gpsimd so slot-waiting loads never delay rp/shift/store
            # issues on the sync ring. ----
            xt, xtf = {}, {}

            def load_half(nm, dten, dk, half):
                t = xpool.tile([128, 1024], FP16, tag="x",
                               name=f"x{nm}0{dk}{half}")
                nc.gpsimd.dma_start(
                    out=t[:],
                    in_=dten[0, dk * 128:(dk + 1) * 128,
                             half * 1024:(half + 1) * 1024])
                xt[(nm, dk, half)] = t

            def load_full(nm, dten, dk):
                t = xfpool.tile([128, T], FP16, tag="xf", name=f"x{nm}1{dk}")
                nc.gpsimd.dma_start(out=t[:],
                                    in_=dten[1, dk * 128:(dk + 1) * 128, :])
                xtf[(nm, dk)] = t

            def xsl(nm, b, dk, lo, w):
                if b == 0:
                    return xt[(nm, dk, lo // 1024)][:, lo % 1024:lo % 1024 + w]
                return xtf[(nm, dk)][:, lo:lo + w]

            srcs = (("q", d_qT), ("k", d_kT), ("v", d_vT))
            for half in range(2):
                for nm, dten in srcs:
                    for dk in range(NDC):
                        load_half(nm, dten, dk, half)
            for nm, dten in srcs:
                for dk in range(NDC):
                    load_full(nm, dten, dk)

            # ---- relpos: rotating pool, prefetched 2 chunks ahead, loaded
            # per batch (sync ring; tiny tiles so never a long slot wait) ----
            rp_tiles = {}

            def rp_load(b, qg, ck):
                t = rppool.tile([128, 2 * QG], FP8, tag="rp",
                                name=f"rp{b}{qg}_{ck}")
                nc.sync.dma_start(out=t[:], in_=d_rp[qg, ck])
                rp_tiles[(b, qg, ck)] = t

            # ---- projection pieces ----
            done = set()

            def proj_qk_piece(nm, b, cc):
                dst = qt_sb[b] if nm == "q" else kt_sb[b]
                acc = miscP.tile([128, QG], F32, tag="mps",
                                 name=f"pa{nm}{b}{cc}")
                for dk in range(NDC):
                    nc.tensor.matmul(
                        acc[:], w_sb[nm][:, dk, :],
                        xsl(nm, b, dk, cc * QG, QG),
                        start=(dk == 0), stop=(dk == NDC - 1))
                nc.vector.tensor_copy(dst[:, cc * QG:(cc + 1) * QG], acc[:])
                done.add((nm, b, cc))

            def proj_v_piece(b, tb):
                acc = miscP.tile([128, 128], F32, tag="mps",
                                 name=f"pav{b}{tb}")
                for dk in range(NDC):
                    nc.tensor.matmul(
                        acc[:], xsl("v", b, dk, tb * 128, 128),
                        w_sb["v"][:, dk, :],
                        start=(dk == 0), stop=(dk == NDC - 1))
                for h in range(HPC):
                    nc.vector.tensor_copy(
                        vaug[(b, h)][:, tb * 80:tb * 80 + DK],
                        acc[:, h * DK:(h + 1) * DK])
                done.add(("v", b, tb))

            # ---- out-projection piece (full 128-contraction) ----
            oc_big = {}

            def oproj_piece(b, qg, db):
                qs = slice(qg * QG, (qg + 1) * QG)
                pp = miscP.tile([128, QG], F32, tag="mps",
                                name=f"pp{b}{qg}{db}")
                nc.tensor.matmul(pp[:], wo_sb[:, db * 128:(db + 1) * 128],
                                 at2[b][:, qs], start=True, stop=True)
                oc = oc_big[(b, qg)]
                nc.vector.tensor_copy(oc[:, db * QG:(db + 1) * QG], pp[:])
                if db == NDC - 1:
                    nc.sync.dma_start(
                        out=d_out[b, :, :, qs],
                        in_=oc[:].rearrange("p (c q) -> p c q", q=QG))

            # ---- normalization (runs when a qg's last P@V has been issued) ----
            def norm(b, qg, ops_t):
                o2 = nrm.tile([DK + 1, 2 * QG], F32, tag="o2",
                              name=f"o2{b}{qg}")
                nc.vector.tensor_copy(o2[:], ops_t[:])
                rc = nrm.tile([DK + 1, 2 * QG], F32, tag="rc",
                              name=f"rc{b}{qg}")
                nc.vector.reciprocal_approx_fast(out=rc[:], in_=o2[:])
                rch = nrm.tile([DK + 1, 2 * QG], FP16, tag="rch",
                               name=f"rch{b}{qg}")
                nc.vector.tensor_copy(rch[:], rc[:])
                rcl = nrm.tile([DK + 1, 2 * QG], FP16, tag="rcl",
                               name=f"rcl{b}{qg}")
                nc.vector.tensor_sub(rcl[:], rc[:], rch[:])
                qs = slice(qg * QG, (qg + 1) * QG)
                for h in range(HPC):
                    hs = slice(h * QG, (h + 1) * QG)
                    rb = miscP.tile([DK, QG], F32, tag="mps",
                                    name=f"rb{b}{qg}{h}")
                    nc.tensor.matmul(rb[:], ones16[DK:DK + 1, :],
                                     rch[DK:DK + 1, hs],
                                     start=True, stop=False)
                    nc.tensor.matmul(rb[:], ones16[DK:DK + 1, :],
                                     rcl[DK:DK + 1, hs],
                                     start=False, stop=True)
                    if h == 0:
                        nc.vector.tensor_mul(at2[b][0:DK, qs],
                                             o2[0:DK, 0:QG], rb[:])
                    else:
                        tmp = tmppool.tile([DK, QG], FP16, tag="tmp",
                                           name=f"tm{b}{qg}")
                        nc.vector.tensor_mul(tmp[:], o2[0:DK, QG:2 * QG],
                                             rb[:])
                        nc.sync.dma_start(out=at2[b][DK:128, qs], in_=tmp[:])
                oc_big[(b, qg)] = ocpool.tile([128, NDC * QG], FP16, tag="oc",
                                              name=f"oc{b}{qg}")
                fillq.extend(("o", b, qg, db) for db in range(NDC))

            # ---- filler scheduling ----
            fillq = deque()

            def emit_item(it):
                if it[0] == "o":
                    oproj_piece(it[1], it[2], it[3])
                elif it[0] == "v":
                    proj_v_piece(it[1], it[2])
                else:
                    proj_qk_piece(it[0], it[1], it[2])

            def fill(n):
                for _ in range(n):
                    if not fillq:
                        return
                    emit_item(fillq.popleft())

            def ensure(b, qg):
                """Force-emit proj pieces this (b, qg) depends on."""
                need = [("q", b, qg)] + [("k", b, cc) for cc in range(qg + 1)]
                need += [("v", b, tb) for tb in range(4 * qg + 4)]
                missing = [k for k in need if k not in done]
                if not missing:
                    return
                miss = set(missing)
                while miss - done and fillq:
                    emit_item(fillq.popleft())

            # ---- P@V with one-chunk lag ----
            pend_pv = []

            def emit_pv():
                b, qg, ck, co, e2, ops_t = pend_pv.pop(0)
                last = (ck == 4 * qg + 3)
                for h in range(HPC):
                    nc.tensor.matmul(
                        ops_t[:, h * QG + co:(h + 1) * QG],
                        vaug[(b, h)][:, ck * 80:ck * 80 + DK + 1],
                        e2[:, h * QG + co:(h + 1) * QG],
                        start=(ck == 0), stop=last)
                if last:
                    norm(b, qg, ops_t)

            # ---- startup: qg0 deps for b=0, first rp prefetches ----
            rp_load(0, *CHUNKS[0])
            rp_load(0, *CHUNKS[1])
            # dummy matmuls: keep the PE busy through the HAM activity
            # window while the first x DMAs land (so real work runs at
            # 2.4 GHz; once warm, sub-3.4us waits do not re-throttle)
            dsrc = persist.tile([1, QG], FP16, tag="dsrc", name="dsrc")
            nc.vector.memset(dsrc[:], 0.0)

            def dummy_mm(n):
                for _ in range(n):
                    dps = miscP.tile([DK, QG], F32, tag="mps", name="dummy")
                    nc.tensor.matmul(dps[:], ones16[0:1, :], dsrc[:],
                                     start=True, stop=True)

            dummy_mm(9)
            proj_qk_piece("q", 0, 0)
            dummy_mm(2)
            proj_qk_piece("k", 0, 0)
            dummy_mm(2)
            for tb in range(4):
                proj_v_piece(0, tb)
                if tb < 2:
                    dummy_mm(1)

            # remaining b0 pieces, deadline-ordered
            for qg in range(1, NQG):
                fillq.append(("k", 0, qg))
                fillq.append(("q", 0, qg))
                fillq.extend(("v", 0, tb) for tb in range(4 * qg, 4 * qg + 4))
            # b1 pieces tensor-major: q fully before k before v, so each
            # full-row x tile's last reader precedes the next tensor's
            # loads in the PE stream (xf slots release in load order --
            # interleaving here deadlocks the gpsimd DMA queue)
            for cc in range(NQG):
                fillq.append(("q", 1, cc))
            for cc in range(NQG):
                fillq.append(("k", 1, cc))
            fillq.extend(("v", 1, tb) for tb in range(NKC))

            # ---- main attention stream ----
            for b in range(B):
                for ci, (qg, ck) in enumerate(CHUNKS):
                    if ck == 0:
                        ensure(b, qg)
                        ops_t = opsP.tile([DK + 1, 2 * QG], F32, tag="ops",
                                          name=f"ops{b}{qg}")
                    if ci + 2 < len(CHUNKS):
                        rp_load(b, *CHUNKS[ci + 2])
                    elif b == 0:
                        rp_load(1, *CHUNKS[ci + 2 - len(CHUNKS)])
                    fill(1 + (len(fillq) > 8))
                    co = max(0, ck * 128 - qg * QG)
                    rp_t = rp_tiles.pop((b, qg, ck))
                    s2 = ps2.tile([128, 2 * QG], F32, tag="s2",
                                  name=f"s2{b}{qg}{ck}")
                    ks = slice(ck * 128, (ck + 1) * 128)
                    q0 = qg * QG + co
                    nc.tensor.matmul(s2[:, co:QG], id8[:], rp_t[:, co:QG],
                                     start=True, stop=False)
                    nc.tensor.matmul(s2[:, QG + co:2 * QG], id8[:],
                                     rp_t[:, QG + co:2 * QG],
                                     start=True, stop=False)
                    nc.tensor.matmul(s2[:, co:QG], kt_sb[b][0:DK, ks],
                                     qt_sb[b][0:DK, q0:(qg + 1) * QG],
                                     start=False, stop=True)
                    nc.tensor.matmul(s2[:, QG + co:2 * QG],
                                     kt_sb[b][DK:128, ks],
                                     qt_sb[b][DK:128, q0:(qg + 1) * QG],
                                     start=False, stop=True)
                    e2 = epool.tile([128, 2 * QG], FP16, tag="e2",
                                    name=f"e2{b}{qg}{ck}")
                    nc.scalar.activation(
                        e2[:], s2[:], mybir.ActivationFunctionType.Exp,
                        bias=kpad[:, b, ck:ck + 1])
                    pend_pv.append((b, qg, ck, co, e2, ops_t))
                    while len(pend_pv) > 1:
                        emit_pv()
            while pend_pv:
                emit_pv()
            while fillq:
                emit_item(fillq.popleft())

    nc.compile()
    return nc


def _prep_host(q, k, v, key_pad_mask, attn_mask, relpos_bias, Wq, Wk, Wv, Wo):
    f32, f16 = np.float32, np.float16
    qT = np.asarray(q, f32).transpose(0, 2, 1).astype(f16)
    kT = np.asarray(k, f32).transpose(0, 2, 1).astype(f16)
    vT = np.asarray(v, f32).transpose(0, 2, 1).astype(f16)

    kb = np.where(np.asarray(key_pad_mask), NEG, f32(0)).astype(f32)  # [B,T]
    kpadT = np.ascontiguousarray(kb.reshape(B, NKC, 128).transpose(2, 0, 1))

    maskT = np.asarray(attn_mask).T  # [k, q], True = masked (k > q)
    rp = np.asarray(relpos_bias, f32)

    id8 = np.eye(128, dtype=ml_dtypes.float8_e4m3)

    def _wmajor(W, rows):  # this core's 128 out-dims -> [128, NDC, 128]
        wT = np.ascontiguousarray(W[rows].T)  # [D, 128]
        return np.ascontiguousarray(
            wT.reshape(NDC, 128, 128).transpose(1, 0, 2)).astype(f16)

    Wq = np.asarray(Wq, f32) * f32(1.0 / np.sqrt(DK))
    Wk = np.asarray(Wk, f32)
    Wv = np.asarray(Wv, f32)
    Wo = np.asarray(Wo, f32)

    in_maps = []
    for c in range(NCORES):
        rows = slice(c * 128, (c + 1) * 128)
        h0 = 2 * c
        rpT = np.where(maskT[None], f32(-240.0),
                       rp[h0:h0 + 2].transpose(0, 2, 1)).astype(
                           ml_dtypes.float8_e4m3)           # [2, Tk, Tq]
        # pack -> [qg, ck, kk, h*512 + qq]
        rp_pack = np.ascontiguousarray(
            rpT.reshape(HPC, NKC, 128, NQG, QG)
               .transpose(3, 1, 2, 0, 4)
               .reshape(NQG, NKC, 128, HPC * QG))
        woT = np.ascontiguousarray(Wo[:, rows].T).astype(f16)  # [128, D]
        in_maps.append({
            "qT": qT, "kT": kT, "vT": vT,
            "relposT": rp_pack,
            "kpadT": kpadT,
            "wqT": _wmajor(Wq, rows),
            "wkT": _wmajor(Wk, rows),
            "wvT": _wmajor(Wv, rows),
            "woT": woT,
            "id8": id8,
        })
    return in_maps


def run(trace=False, tmpdir=None, **inputs):
    if "nc" not in _CACHE:
        _CACHE["nc"] = _build_program()
    nc = _CACHE["nc"]
    in_maps = _prep_host(**inputs)
    res = run_bass_kernel_spmd(nc, in_maps, core_ids=list(range(NCORES)),
                               trace=trace, tmpdir=tmpdir)
    acc = res.results[0]["outT"].astype(np.float64)
    for c in range(1, NCORES):
        acc += res.results[c]["outT"]
    # [B, 128, NDC, T] -> [B, T, D] with d = c*128 + p
    out = acc.transpose(0, 2, 1, 3).reshape(B, D, T)
    out = np.ascontiguousarray(out.transpose(0, 2, 1)).astype(np.float32)
    return out, res


def kernel(**inputs) -> np.ndarray:
    out, _ = run(trace=False, **inputs)
    return out
